# revision 4
# baseline (speedup 1.0000x reference)
"""Multi-head dot-product attention (B=2, Q=K=2048, EMB=2048, H=32, D=64) on 8 TRN2 cores.

Sharding: data parallel over batch (2) x tensor parallel over heads (4 groups of 8).
Core c handles batch c//4, heads 8*(c%4) .. 8*(c%4)+8. Each core computes a partial
output [2048, 2048] (its heads' contribution through wo); host sums the 4 head-group
partials per batch.

v2: single fully-interleaved emission so the PE never idles; the scalar engine's
exp stream (~294us) and all DVE/DMA work hide under the PE's ~437us of bf16
matmul streaming. exp(bias^T) is precomputed on the host (expbT input).

Emission order (PE work in brackets):
  fill:     [K-proj all, V-proj tc0-1, Q-proj qc0]   (xkv/xq streamed, 2 slots)
  attn qc0: per pair, software-pipelined kc loop [scores, ctx]; V-proj tc2-3
            re-streamed and emitted just-in-time inside pair0's kc loop
  attn qc1: per pair [q-proj qc1] + kc loop; out-proj of qc0 token rows
            sprinkled into the kc loops
  tail:     [out-proj qc1 token rows]
"""

import numpy as np
import ml_dtypes
from contextlib import ExitStack

import concourse.bass as bass
from concourse import bacc
import concourse.mybir as mybir
import concourse.tile as tile
from concourse.bass_utils import run_bass_kernel_spmd

BF16 = mybir.dt.bfloat16
F32 = mybir.dt.float32
AF = mybir.ActivationFunctionType

B, T, E = 2, 2048, 2048          # batch, tokens (Q=K), embed
H, D = 32, 64                     # total heads, head dim
NH = 8                            # heads per core
HD = NH * D                       # 512, per-core head-dim total
EC = E // 128                     # 16 contraction chunks
KC = T // 128                     # 16 key chunks
QCH = 1024                        # attention q-chunk (psum + exp tile width)
NQC = T // QCH                    # 2
N_CORES = 8


def build_program():
    nc = bacc.Bacc("TRN2", target_bir_lowering=False, debug=False,
                   num_devices=N_CORES)

    xqT = nc.dram_tensor("xqT", [E, T], BF16, kind="ExternalInput").ap()
    xkvT = nc.dram_tensor("xkvT", [E, T], BF16, kind="ExternalInput").ap()
    expbT = nc.dram_tensor("expbT", [T, T], BF16, kind="ExternalInput").ap()
    wq = nc.dram_tensor("wq", [E, HD], BF16, kind="ExternalInput").ap()
    wk = nc.dram_tensor("wk", [E, HD], BF16, kind="ExternalInput").ap()
    wv = nc.dram_tensor("wv", [E, HD], BF16, kind="ExternalInput").ap()
    wo = nc.dram_tensor("wo", [HD, E], BF16, kind="ExternalInput").ap()
    out = nc.dram_tensor("out", [T, E], F32, kind="ExternalOutput").ap()

    with tile.TileContext(nc) as tc, ExitStack() as ctx:
        persist = ctx.enter_context(tc.tile_pool(name="persist", bufs=1))
        qT_sb = persist.tile([128, HD // 128, T], BF16)   # q^T[hd, t]
        kT_sb = persist.tile([128, HD // 128, T], BF16)
        v_sb = persist.tile([128, KC, NH, D + 1], BF16)   # v[k, h, d] + ones
        ctxT_sb = persist.tile([128, HD // 128, T], BF16)
        nc.vector.memset(v_sb[:, :, :, D:D + 1], 1.0)

        # wk's slot is reused for wo later (same tag)
        wpool = ctx.enter_context(tc.tile_pool(name="wpool", bufs=1))
        wk_sb = wpool.tile([128, EC, HD], BF16, tag="wkwo", name="wk_sb")
        wq_sb = wpool.tile([128, EC, HD], BF16, tag="wq", name="wq_sb")
        wv_sb = wpool.tile([128, EC, HD], BF16, tag="wv", name="wv_sb")

        expbp = ctx.enter_context(tc.tile_pool(name="expbp", bufs=1))
        xsp = ctx.enter_context(tc.tile_pool(name="xsp", bufs=2))
        psp = ctx.enter_context(tc.tile_pool(name="psp", bufs=2, space="PSUM"))
        atp = ctx.enter_context(tc.tile_pool(name="atp", bufs=2))
        a2p = ctx.enter_context(tc.tile_pool(name="a2p", bufs=2))
        nrm = ctx.enter_context(tc.tile_pool(name="nrm", bufs=2))
        nrmd = ctx.enter_context(tc.tile_pool(name="nrmd", bufs=2, space="DRAM"))

        # ---------------- emission helpers ----------------
        def x_stream(src, tc4):
            """Stream activation chunk [128, EC, 512] for token cols tc4*512.."""
            xs = xsp.tile([128, EC, 512], BF16, name="xs", tag="xs")
            nc.sync.dma_start(
                out=xs[:],
                in_=bass.AP(tensor=src.tensor, offset=src.offset + tc4 * 512,
                            ap=[[T, 128], [128 * T, EC], [1, 512]]))
            return xs

        def proj_tile(w_sb, x_sb, dst, hdc, tc4):
            """One [128,512] psum tile of a w^T @ x projection -> dst slice."""
            ps = psp.tile([128, QCH], F32, tag="s", name="ps")
            for ec in range(EC):
                nc.tensor.matmul(ps[:, 0:512],
                                 lhsT=w_sb[:, ec, hdc * 128:(hdc + 1) * 128],
                                 rhs=x_sb[:, ec, :],
                                 start=(ec == 0), stop=(ec == EC - 1))
            nc.vector.tensor_copy(dst[:, hdc, tc4 * 512:(tc4 + 1) * 512],
                                  ps[:, 0:512])

        def v_tiles(x_sb, tc4):
            """V-projection tiles for key chunk tc4 (kc = 4*tc4 .. 4*tc4+3)."""
            for sub in range(4):
                kc = tc4 * 4 + sub
                ps = psp.tile([128, QCH], F32, tag="s", name="vps")
                for ec in range(EC):
                    nc.tensor.matmul(ps[:, 0:512],
                                     lhsT=x_sb[:, ec, sub * 128:(sub + 1) * 128],
                                     rhs=wv_sb[:, ec, :],
                                     start=(ec == 0), stop=(ec == EC - 1))
                nc.vector.tensor_copy(
                    v_sb[:, kc, :, 0:D],
                    ps[:, 0:512].rearrange("p (h d) -> p h d", h=NH))

        def expb_tile(kc, qc):
            eb = expbp.tile([128, QCH], BF16, tag=f"e{kc}", name=f"eb{kc}")
            nc.gpsimd.dma_start(
                out=eb[:],
                in_=expbT[kc * 128:(kc + 1) * 128, qc * QCH:(qc + 1) * QCH])
            return eb

        wo_ref = [None]

        def out_tile(i):
            """Out-projection psum tile i (of 32): tc16 = i//2, e-half = i%2."""
            tc16, eh = i // 2, i % 2
            po = psp.tile([128, QCH], F32, tag="s", name="po")
            for hdc in range(HD // 128):
                for ncol in range(2):
                    nc.tensor.matmul(
                        po[:, ncol * 512:(ncol + 1) * 512],
                        lhsT=ctxT_sb[:, hdc, tc16 * 128:(tc16 + 1) * 128],
                        rhs=wo_ref[0][:, hdc, eh * 1024 + ncol * 512:
                                      eh * 1024 + (ncol + 1) * 512],
                        start=(hdc == 0), stop=(hdc == HD // 128 - 1),
                        skip_group_check=True)
            ot = nrm.tile([128, QCH], F32, tag="ostage", name="ot")
            nc.vector.tensor_copy(ot[:], po[:])
            nc.sync.dma_start(
                out=out[tc16 * 128:(tc16 + 1) * 128,
                        eh * 1024:(eh + 1) * 1024],
                in_=ot[:])

        def emit_scores(pair, qc, kc):
            tiles = []
            for hh in range(2):
                pr = slice(hh * D, (hh + 1) * D)
                s = psp.tile([128, QCH], F32, tag="s", name="s")
                for half in range(QCH // 512):
                    q0 = qc * QCH + half * 512
                    nc.tensor.matmul(
                        s[:, half * 512:(half + 1) * 512],
                        lhsT=kT_sb[pr, pair, kc * 128:(kc + 1) * 128],
                        rhs=qT_sb[pr, pair, q0:q0 + 512],
                        start=True, stop=True)
                tiles.append(s)
            return tiles

        def emit_expmul(s_tiles, eb):
            a2s = []
            for hh in range(2):
                at = atp.tile([128, QCH], BF16, tag="at", name="at")
                nc.scalar.activation(at[:], s_tiles[hh][:], AF.Exp)
                a2 = a2p.tile([128, QCH], BF16, tag="a2", name="a2")
                nc.vector.tensor_mul(a2[:], at[:], eb[:])
                a2s.append(a2)
            return a2s

        def emit_ctx(ctx_t, pair, kc, a2s):
            for hh in range(2):
                h = pair * 2 + hh
                for half in range(QCH // 512):
                    nc.tensor.matmul(
                        ctx_t[hh][:, half * 512:(half + 1) * 512],
                        lhsT=v_sb[:, kc, h, :],
                        rhs=a2s[hh][:, half * 512:(half + 1) * 512],
                        start=(kc == 0), stop=(kc == KC - 1))

        def norm_pair(ctx_t, pair, qc):
            """Normalize both heads of a pair: ctx/rowsum -> ctxT_sb."""
            for hh in range(2):
                ctxf = nrm.tile([D + 1, QCH], BF16, tag="ctxf", name="ctxf")
                nc.vector.tensor_copy(ctxf[:], ctx_t[hh][:])  # frees psum slot
                # rowsum row -> [128, 8] partition-major scratch (DMA casts)
                srow = nrm.tile([128, QCH // 128], F32, tag="srow", name="srow")
                nc.gpsimd.dma_start(out=srow[:], in_=ctxf[D:D + 1, :])
                rec = nrm.tile([128, QCH // 128], F32, tag="rec", name="rec")
                nc.vector.reciprocal_approx_fast(out=rec[:], in_=srow[:])
                rec_d = nrmd.tile([QCH], F32, tag="recd", name="recd")
                nc.sync.dma_start(
                    out=rec_d[:].rearrange("(p j) -> p j", p=128), in_=rec[:])
                recb = nrm.tile([D, QCH], BF16, tag="recb", name="recb")
                rd = rec_d[:]
                bcast = bass.AP(tensor=rd.tensor, offset=rd.offset,
                                ap=[[0, D]] + list(rd.ap))
                nc.gpsimd.dma_start(out=recb[:], in_=bcast)  # casts f32->bf16
                if hh == 0:
                    nc.vector.tensor_mul(
                        ctxT_sb[0:D, pair, qc * QCH:(qc + 1) * QCH],
                        ctxf[0:D, :], recb[:])
                else:
                    stage = nrm.tile([D, QCH], BF16, tag="cstage", name="stg")
                    nc.vector.tensor_mul(stage[:], ctxf[0:D, :], recb[:])
                    nc.sync.dma_start(
                        out=ctxT_sb[D:2 * D, pair, qc * QCH:(qc + 1) * QCH],
                        in_=stage[:])

        # ---------------- fill phase ----------------
        nc.gpsimd.dma_start(out=wk_sb[:],
                            in_=wk.rearrange("(ec p) n -> p ec n", p=128))
        nc.gpsimd.dma_start(out=wv_sb[:],
                            in_=wv.rearrange("(ec p) n -> p ec n", p=128))
        nc.gpsimd.dma_start(out=wq_sb[:],
                            in_=wq.rearrange("(ec p) n -> p ec n", p=128))
        expb_tiles = {kc: expb_tile(kc, 0) for kc in range(KC)}

        for tc4 in range(4):
            xs = x_stream(xkvT, tc4)
            for hdc in range(4):
                proj_tile(wk_sb, xs, kT_sb, hdc, tc4)
            if tc4 < 2:
                v_tiles(xs, tc4)     # V for kc 0..7
        for tc4 in range(2):
            xs = x_stream(xqT, tc4)
            for hdc in range(4):
                proj_tile(wq_sb, xs, qT_sb, hdc, tc4)

        # re-streams for deferred V chunks tc2/tc3 (issued now, used in-loop)
        xkv_late = {tc4: x_stream(xkvT, tc4) for tc4 in (2, 3)}
        # wo reuses wk's slot; its DMA waits on the last wk reader
        wo_sb = wpool.tile([128, HD // 128, E], BF16, tag="wkwo", name="wo_sb")
        nc.gpsimd.dma_start(out=wo_sb[:],
                            in_=wo.rearrange("(c p) n -> p c n", p=128))
        wo_ref[0] = wo_sb

        # ---------------- attention ----------------
        xq_late = {}
        out_i = 0
        for qc in range(NQC):
            if qc == 1:
                for tc4 in (2, 3):
                    xq_late[tc4] = x_stream(xqT, tc4)
            for pair in range(NH // 2):
                if qc == 1:
                    for tc4 in (2, 3):
                        proj_tile(wq_sb, xq_late[tc4], qT_sb, pair, tc4)
                ctx_t = [psp.tile([D + 1, QCH], F32, tag="ctx", name=f"ctx{hh}")
                         for hh in range(2)]
                s_cur = emit_scores(pair, qc, 0)
                for kc in range(KC):
                    a2s = emit_expmul(s_cur, expb_tiles[kc])
                    if qc == 0 and pair == 3:
                        # reload this kc's expb slot for qc1 (last reader above)
                        expb_tiles[kc] = expb_tile(kc, 1)
                    if kc < KC - 1:
                        s_cur = emit_scores(pair, qc, kc + 1)
                    emit_ctx(ctx_t, pair, kc, a2s)
                    # interleaved independent PE work, just-in-time:
                    if qc == 0 and pair == 0 and kc in (5, 9):
                        v_tiles(xkv_late[kc // 4 + 1], kc // 4 + 1)
                    if qc == 1 and kc in (5, 13):
                        out_tile(out_i); out_i += 1
                        out_tile(out_i); out_i += 1
                norm_pair(ctx_t, pair, qc)

        # ---------------- tail: out-proj for qc1 rows ----------------
        for i in range(out_i, 32):
            out_tile(i)

    nc.compile()
    return nc


_NC_CACHE = {}


def kernel(inputs_q, inputs_kv, bias, wq, wk, wv, wo):
    bf16 = ml_dtypes.bfloat16
    inputs_q = np.asarray(inputs_q)
    inputs_kv = np.asarray(inputs_kv)
    bias = np.asarray(bias)
    # fold the reference's 1/sqrt(D) query scaling into wq
    wq_s = (np.asarray(wq).reshape(E, H * D) / np.sqrt(D)).astype(bf16)
    wk_s = np.asarray(wk).reshape(E, H * D).astype(bf16)
    wv_s = np.asarray(wv).reshape(E, H * D).astype(bf16)
    wo_s = np.asarray(wo).reshape(H * D, E).astype(bf16)

    # host-side layout marshaling: embed-major activations, key-major exp(bias)
    xq_b = [np.ascontiguousarray(inputs_q[b].T).astype(bf16) for b in range(B)]
    xkv_b = [np.ascontiguousarray(inputs_kv[b].T).astype(bf16) for b in range(B)]
    expb_b = [np.exp(np.ascontiguousarray(bias[b, 0].T)).astype(bf16)
              for b in range(B)]

    in_maps = []
    for c in range(N_CORES):
        b, hg = c // 4, c % 4
        hs = slice(hg * HD, (hg + 1) * HD)
        in_maps.append({
            "xqT": xq_b[b],
            "xkvT": xkv_b[b],
            "expbT": expb_b[b],
            "wq": np.ascontiguousarray(wq_s[:, hs]),
            "wk": np.ascontiguousarray(wk_s[:, hs]),
            "wv": np.ascontiguousarray(wv_s[:, hs]),
            "wo": np.ascontiguousarray(wo_s[hs, :]),
        })

    if "nc" not in _NC_CACHE:
        _NC_CACHE["nc"] = build_program()
    nc = _NC_CACHE["nc"]

    res = run_bass_kernel_spmd(nc, in_maps, list(range(N_CORES)))
    outs = [np.asarray(r["out"], dtype=np.float32) for r in res.results]
    full = np.empty((B, T, E), dtype=np.float32)
    for b in range(B):
        full[b] = outs[4 * b] + outs[4 * b + 1] + outs[4 * b + 2] + outs[4 * b + 3]
    return full


# revision 5
# speedup vs baseline: 1.0484x; 1.0484x over previous
"""Multi-head dot-product attention (B=2, Q=K=2048, EMB=2048, H=32, D=64) on 8 TRN2 cores.

Sharding: data parallel over batch (2) x tensor parallel over heads (4 groups of 8).
Core c handles batch c//4, heads 8*(c%4) .. 8*(c%4)+8. Each core computes a partial
output [2048, 2048] (its heads' contribution through wo) in bf16; host sums the 4
head-group partials per batch in f32.

v3: fully-interleaved emission tuned so the PE never idles (idle also drops the
PE out of its max p-state, doubling matmul time for ~3us):
  - software-pipelined attention: scores run 2 kc ahead, ctx lags 1 kc, so the
    scores->exp->mul->ctx round trip (~2.3us) never blocks the in-order PE queue
  - independent PE work (V-proj tc2/3, Q-proj qc1, out-proj qc0) is sprinkled
    one psum-tile at a time into the kc loops to cover the scalar-vs-PE rate gap
  - exp(bias^T) precomputed on host (expbT input); first weight/stream DMAs
    chunked so fill matmuls start early
"""

import numpy as np
import ml_dtypes
from contextlib import ExitStack

import concourse.bass as bass
from concourse import bacc
import concourse.mybir as mybir
import concourse.tile as tile
from concourse.bass_utils import run_bass_kernel_spmd

BF16 = mybir.dt.bfloat16
F32 = mybir.dt.float32
AF = mybir.ActivationFunctionType

B, T, E = 2, 2048, 2048          # batch, tokens (Q=K), embed
H, D = 32, 64                     # total heads, head dim
NH = 8                            # heads per core
HD = NH * D                       # 512, per-core head-dim total
EC = E // 128                     # 16 contraction chunks
KC = T // 128                     # 16 key chunks
QCH = 1024                        # attention q-chunk (psum + exp tile width)
NQC = T // QCH                    # 2
N_CORES = 8


def build_program():
    nc = bacc.Bacc("TRN2", target_bir_lowering=False, debug=False,
                   num_devices=N_CORES)

    xqT = nc.dram_tensor("xqT", [E, T], BF16, kind="ExternalInput").ap()
    xkvT = nc.dram_tensor("xkvT", [E, T], BF16, kind="ExternalInput").ap()
    expbT = nc.dram_tensor("expbT", [T, T], BF16, kind="ExternalInput").ap()
    wq = nc.dram_tensor("wq", [E, HD], BF16, kind="ExternalInput").ap()
    wk = nc.dram_tensor("wk", [E, HD], BF16, kind="ExternalInput").ap()
    wv = nc.dram_tensor("wv", [E, HD], BF16, kind="ExternalInput").ap()
    wo = nc.dram_tensor("wo", [HD, E], BF16, kind="ExternalInput").ap()
    out = nc.dram_tensor("out", [T, E], BF16, kind="ExternalOutput").ap()

    with tile.TileContext(nc) as tc, ExitStack() as ctx:
        persist = ctx.enter_context(tc.tile_pool(name="persist", bufs=1))
        qT_sb = persist.tile([128, HD // 128, T], BF16)   # q^T[hd, t]
        kT_sb = persist.tile([128, HD // 128, T], BF16)
        v_sb = persist.tile([128, KC, NH, D + 1], BF16)   # v[k, h, d] + ones
        ctxT_sb = persist.tile([128, HD // 128, T], BF16)
        nc.vector.memset(v_sb[:, :, :, D:D + 1], 1.0)

        # wk's slot is reused for wo later (same tag)
        wpool = ctx.enter_context(tc.tile_pool(name="wpool", bufs=1))
        wk_sb = wpool.tile([128, EC, HD], BF16, tag="wkwo", name="wk_sb")
        wq_sb = wpool.tile([128, EC, HD], BF16, tag="wq", name="wq_sb")
        wv_sb = wpool.tile([128, EC, HD], BF16, tag="wv", name="wv_sb")

        expbp = ctx.enter_context(tc.tile_pool(name="expbp", bufs=1))
        xsp = ctx.enter_context(tc.tile_pool(name="xsp", bufs=2))
        psp = ctx.enter_context(tc.tile_pool(name="psp", bufs=2, space="PSUM"))
        atp = ctx.enter_context(tc.tile_pool(name="atp", bufs=3))
        a2p = ctx.enter_context(tc.tile_pool(name="a2p", bufs=4))
        nrm = ctx.enter_context(tc.tile_pool(name="nrm", bufs=2))
        nrmd = ctx.enter_context(tc.tile_pool(name="nrmd", bufs=2, space="DRAM"))

        # ---------------- emission helpers ----------------
        def dma_chunked(dst, src_re, nchunk):
            """Chunk a [128, EC, HD]-style weight DMA along dim1 so early
            matmuls can start before the full tensor lands."""
            step = EC // nchunk
            for i in range(nchunk):
                nc.gpsimd.dma_start(out=dst[:, i * step:(i + 1) * step, :],
                                    in_=src_re[:, i * step:(i + 1) * step, :])

        def x_stream(src, tc4, nchunk=1, q=None):
            """Stream activation chunk [128, EC, 512] for token cols tc4*512.."""
            xs = xsp.tile([128, EC, 512], BF16, name="xs", tag="xs")
            step = EC // nchunk
            for i in range(nchunk):
                (q or nc.sync).dma_start(
                    out=xs[:, i * step:(i + 1) * step, :],
                    in_=bass.AP(tensor=src.tensor,
                                offset=src.offset + tc4 * 512 + i * step * 128 * T,
                                ap=[[T, 128], [128 * T, step], [1, 512]]))
            return xs

        def proj_tile(w_sb, x_sb, dst, hdc, tc4):
            """One [128,512] psum tile of a w^T @ x projection -> dst slice."""
            ps = psp.tile([128, QCH], F32, tag="s", name="ps")
            for ec in range(EC):
                nc.tensor.matmul(ps[:, 0:512],
                                 lhsT=w_sb[:, ec, hdc * 128:(hdc + 1) * 128],
                                 rhs=x_sb[:, ec, :],
                                 start=(ec == 0), stop=(ec == EC - 1))
            nc.vector.tensor_copy(dst[:, hdc, tc4 * 512:(tc4 + 1) * 512],
                                  ps[:, 0:512])

        def v_tile(x_sb, tc4, sub):
            """One V-projection psum tile: v rows for key chunk kc=4*tc4+sub."""
            kc = tc4 * 4 + sub
            ps = psp.tile([128, QCH], F32, tag="s", name="vps")
            for ec in range(EC):
                nc.tensor.matmul(ps[:, 0:512],
                                 lhsT=x_sb[:, ec, sub * 128:(sub + 1) * 128],
                                 rhs=wv_sb[:, ec, :],
                                 start=(ec == 0), stop=(ec == EC - 1))
            nc.vector.tensor_copy(
                v_sb[:, kc, :, 0:D],
                ps[:, 0:512].rearrange("p (h d) -> p h d", h=NH))

        def expb_tile(kc, qc):
            eb = expbp.tile([128, QCH], BF16, tag=f"e{kc}", name=f"eb{kc}")
            nc.gpsimd.dma_start(
                out=eb[:],
                in_=expbT[kc * 128:(kc + 1) * 128, qc * QCH:(qc + 1) * QCH])
            return eb

        wo_ref = [None]

        def out_tile(i, copy_eng="v"):
            """Out-projection psum tile i (of 32): tc16 = i//2, e-half = i%2."""
            tc16, eh = i // 2, i % 2
            po = psp.tile([128, QCH], F32, tag="s", name="po")
            for hdc in range(HD // 128):
                for ncol in range(2):
                    nc.tensor.matmul(
                        po[:, ncol * 512:(ncol + 1) * 512],
                        lhsT=ctxT_sb[:, hdc, tc16 * 128:(tc16 + 1) * 128],
                        rhs=wo_ref[0][:, hdc, eh * 1024 + ncol * 512:
                                      eh * 1024 + (ncol + 1) * 512],
                        start=(hdc == 0), stop=(hdc == HD // 128 - 1),
                        skip_group_check=True)
            ot = nrm.tile([128, QCH], BF16, tag="ostage", name="ot")
            if copy_eng == "v":
                nc.vector.tensor_copy(ot[:], po[:])
            else:
                nc.scalar.activation(ot[:], po[:], AF.Copy)
            nc.sync.dma_start(
                out=out[tc16 * 128:(tc16 + 1) * 128,
                        eh * 1024:(eh + 1) * 1024],
                in_=ot[:])

        def emit_scores(pair, qc, kc):
            tiles = []
            for hh in range(2):
                pr = slice(hh * D, (hh + 1) * D)
                s = psp.tile([128, QCH], F32, tag="s", name="s")
                for half in range(QCH // 512):
                    q0 = qc * QCH + half * 512
                    nc.tensor.matmul(
                        s[:, half * 512:(half + 1) * 512],
                        lhsT=kT_sb[pr, pair, kc * 128:(kc + 1) * 128],
                        rhs=qT_sb[pr, pair, q0:q0 + 512],
                        start=True, stop=True)
                tiles.append(s)
            return tiles

        def emit_expmul(s_tiles, eb):
            a2s = []
            for hh in range(2):
                at = atp.tile([128, QCH], BF16, tag="at", name="at")
                nc.scalar.activation(at[:], s_tiles[hh][:], AF.Exp)
                a2 = a2p.tile([128, QCH], BF16, tag="a2", name="a2")
                nc.vector.tensor_mul(a2[:], at[:], eb[:])
                a2s.append(a2)
            return a2s

        def emit_ctx(ctx_t, pair, kc, a2s):
            for hh in range(2):
                h = pair * 2 + hh
                for half in range(QCH // 512):
                    nc.tensor.matmul(
                        ctx_t[hh][:, half * 512:(half + 1) * 512],
                        lhsT=v_sb[:, kc, h, :],
                        rhs=a2s[hh][:, half * 512:(half + 1) * 512],
                        start=(kc == 0), stop=(kc == KC - 1))

        def norm_pair(ctx_t, pair, qc):
            """Normalize both heads of a pair: ctx/rowsum -> ctxT_sb."""
            for hh in range(2):
                ctxf = nrm.tile([D + 1, QCH], BF16, tag="ctxf", name="ctxf")
                nc.vector.tensor_copy(ctxf[:], ctx_t[hh][:])  # frees psum slot
                # rowsum row -> [128, 8] partition-major scratch (DMA casts)
                srow = nrm.tile([128, QCH // 128], F32, tag="srow", name="srow")
                nc.gpsimd.dma_start(out=srow[:], in_=ctxf[D:D + 1, :])
                rec = nrm.tile([128, QCH // 128], F32, tag="rec", name="rec")
                nc.vector.reciprocal_approx_fast(out=rec[:], in_=srow[:])
                rec_d = nrmd.tile([QCH], F32, tag="recd", name="recd")
                nc.sync.dma_start(
                    out=rec_d[:].rearrange("(p j) -> p j", p=128), in_=rec[:])
                recb = nrm.tile([D, QCH], BF16, tag="recb", name="recb")
                rd = rec_d[:]
                bcast = bass.AP(tensor=rd.tensor, offset=rd.offset,
                                ap=[[0, D]] + list(rd.ap))
                nc.gpsimd.dma_start(out=recb[:], in_=bcast)  # casts f32->bf16
                if hh == 0:
                    nc.vector.tensor_mul(
                        ctxT_sb[0:D, pair, qc * QCH:(qc + 1) * QCH],
                        ctxf[0:D, :], recb[:])
                else:
                    stage = nrm.tile([D, QCH], BF16, tag="cstage", name="stg")
                    nc.vector.tensor_mul(stage[:], ctxf[0:D, :], recb[:])
                    nc.sync.dma_start(
                        out=ctxT_sb[D:2 * D, pair, qc * QCH:(qc + 1) * QCH],
                        in_=stage[:])

        # ---------------- fill phase ----------------
        dma_chunked(wk_sb, wk.rearrange("(ec p) n -> p ec n", p=128), 4)
        nc.gpsimd.dma_start(out=wv_sb[:],
                            in_=wv.rearrange("(ec p) n -> p ec n", p=128))
        dma_chunked(wq_sb, wq.rearrange("(ec p) n -> p ec n", p=128), 4)
        expb_tiles = {kc: expb_tile(kc, 0) for kc in range(KC)}

        for tc4 in range(4):
            xs = x_stream(xkvT, tc4, nchunk=(4 if tc4 == 0 else 1))
            for hdc in range(4):
                proj_tile(wk_sb, xs, kT_sb, hdc, tc4)
            if tc4 < 2:
                for sub in range(4):
                    v_tile(xs, tc4, sub)     # V for kc 0..7
        for tc4 in range(2):
            xs = x_stream(xqT, tc4)
            for hdc in range(4):
                proj_tile(wq_sb, xs, qT_sb, hdc, tc4)

        # re-streams for deferred V chunks tc2/tc3 (issued now, used in-loop)
        xkv_late = {tc4: x_stream(xkvT, tc4) for tc4 in (2, 3)}
        # wo reuses wk's slot; its DMA waits on the last wk reader
        wo_sb = wpool.tile([128, HD // 128, E], BF16, tag="wkwo", name="wo_sb")
        nc.gpsimd.dma_start(out=wo_sb[:],
                            in_=wo.rearrange("(c p) n -> p c n", p=128))
        wo_ref[0] = wo_sb

        # interleave schedule: (qc, pair, kc) -> list of thunks
        xq_late = {}

        def interleave(qc, pair, kc):
            if qc == 0:
                if pair == 0:
                    # V chunks tc2/tc3, just-in-time for ctx at kc>=8
                    vmap = {3: (2, 0), 5: (2, 1), 7: (2, 2), 9: (2, 3),
                            10: (3, 0), 11: (3, 1), 12: (3, 2), 13: (3, 3)}
                    if kc in vmap:
                        tc4, sub = vmap[kc]
                        v_tile(xkv_late[tc4], tc4, sub)
                    if kc == 15:
                        for tc4 in (2, 3):
                            xq_late[tc4] = x_stream(xqT, tc4)
                else:
                    # q-proj for qc1: 8 tiles spread over pairs 1-3
                    qmap = {(1, 8): 0, (1, 12): 1,
                            (2, 2): 2, (2, 7): 3, (2, 12): 4,
                            (3, 2): 5, (3, 7): 6, (3, 12): 7}
                    ti = qmap.get((pair, kc))
                    if ti is not None:
                        proj_tile(wq_sb, xq_late[2 + ti % 2], qT_sb,
                                  ti // 2, 2 + ti % 2)
            else:
                # out-proj for qc0 token rows: 16 tiles
                if kc in (3, 7, 11, 14):
                    i = pair * 4 + (3, 7, 11, 14).index(kc)
                    out_tile(i)

        # ---------------- attention ----------------
        out_emitted = 16
        for qc in range(NQC):
            for pair in range(NH // 2):
                ctx_t = [psp.tile([D + 1, QCH], F32, tag="ctx",
                                  name=f"ctx{hh}") for hh in range(2)]
                s_tiles = {0: emit_scores(pair, qc, 0),
                           1: emit_scores(pair, qc, 1)}
                a2_hist = {}
                for kc in range(KC):
                    a2_hist[kc] = emit_expmul(s_tiles.pop(kc), expb_tiles[kc])
                    if qc == 0 and pair == 3:
                        # reload this kc's expb slot for qc1 (last reader above)
                        expb_tiles[kc] = expb_tile(kc, 1)
                    if kc + 2 < KC:
                        s_tiles[kc + 2] = emit_scores(pair, qc, kc + 2)
                    interleave(qc, pair, kc)
                    if kc >= 1:
                        emit_ctx(ctx_t, pair, kc - 1, a2_hist.pop(kc - 1))
                emit_ctx(ctx_t, pair, KC - 1, a2_hist.pop(KC - 1))
                norm_pair(ctx_t, pair, qc)

        # ---------------- tail: out-proj for qc1 rows ----------------
        for i in range(out_emitted, 32):
            out_tile(i, copy_eng=("s" if i % 2 else "v"))

    nc.compile()
    return nc


_NC_CACHE = {}


def kernel(inputs_q, inputs_kv, bias, wq, wk, wv, wo):
    bf16 = ml_dtypes.bfloat16
    inputs_q = np.asarray(inputs_q)
    inputs_kv = np.asarray(inputs_kv)
    bias = np.asarray(bias)
    # fold the reference's 1/sqrt(D) query scaling into wq
    wq_s = (np.asarray(wq).reshape(E, H * D) / np.sqrt(D)).astype(bf16)
    wk_s = np.asarray(wk).reshape(E, H * D).astype(bf16)
    wv_s = np.asarray(wv).reshape(E, H * D).astype(bf16)
    wo_s = np.asarray(wo).reshape(H * D, E).astype(bf16)

    # host-side layout marshaling: embed-major activations, key-major exp(bias)
    xq_b = [np.ascontiguousarray(inputs_q[b].T).astype(bf16) for b in range(B)]
    xkv_b = [np.ascontiguousarray(inputs_kv[b].T).astype(bf16) for b in range(B)]
    expb_b = [np.exp(np.ascontiguousarray(bias[b, 0].T)).astype(bf16)
              for b in range(B)]

    in_maps = []
    for c in range(N_CORES):
        b, hg = c // 4, c % 4
        hs = slice(hg * HD, (hg + 1) * HD)
        in_maps.append({
            "xqT": xq_b[b],
            "xkvT": xkv_b[b],
            "expbT": expb_b[b],
            "wq": np.ascontiguousarray(wq_s[:, hs]),
            "wk": np.ascontiguousarray(wk_s[:, hs]),
            "wv": np.ascontiguousarray(wv_s[:, hs]),
            "wo": np.ascontiguousarray(wo_s[hs, :]),
        })

    if "nc" not in _NC_CACHE:
        _NC_CACHE["nc"] = build_program()
    nc = _NC_CACHE["nc"]

    res = run_bass_kernel_spmd(nc, in_maps, list(range(N_CORES)))
    outs = [np.asarray(r["out"], dtype=np.float32) for r in res.results]
    full = np.empty((B, T, E), dtype=np.float32)
    for b in range(B):
        full[b] = outs[4 * b] + outs[4 * b + 1] + outs[4 * b + 2] + outs[4 * b + 3]
    return full
